# revision 31
# baseline (speedup 1.0000x reference)
"""MoE layer (N=8192, D=1024, F=4096, E=8, top-2) on 8 Trainium2 NeuronCores.

Strategy (F-split, fully load-balanced):
  - Host: gate (inputs @ Wg + bg), top-k selection, softmax combine weights,
    token gather per expert (the tiny O(N*D*E) part), final partial-sum
    combine + scatter-add + b2 term.
  - Device (SPMD): every core holds a 512-wide slice of the FFN hidden dim
    (F/8) of ALL 8 experts' weights resident in SBUF (bf16, 128 KB/part)
    and processes ALL token-expert pairs at 1/8 width:
        part_y = cw * (silu(x @ W1[e][:, cut]) + b1[e][cut]) @ W2[e][cut, :]
    mm2 contracts over F, so the 8 per-core partials simply sum on the
    host -- no cross-core communication, and the load is perfectly
    balanced regardless of routing (every core does identical work).

Per-core kernel layout (all flat [128, cols] SBUF tiles, bf16):
  w1: col = e*4096 + ft*1024 + d*128 + f    (stationary [128d x 128f] tiles)
  w2: col = e*4096 + ft*1024 + dcol         (stationary [128f x 128d] tiles)
  x:  per token block (<=512 tokens, single expert): col = d*blk + t
  mm1: h[f, t] = silu(sum_d w1 tile.T @ x tile + b1)    (psum [128f, blk])
  mm2: y[d, t] = sum_ft w2 tile.T @ h[ft block]         (psum [128d, blk])
  copy psum -> bf16 on VectorE, DMA out d-major ([8, 128, T] planes).
mm2 keeps tokens on the moving axis so its cost tracks the exact token
count (no 128-token tile quantization); the combine weight is applied on
the host during the partial-sum reduction. Blocks are interleaved
mm1(b+1) then mm2(b) so the PE never waits on the silu of its own block;
weights/x stream in first-use order (w2 staggered one block after w1) so
the first matmul only gates on ~1.5 MB of DMA.
"""

import math
import os
import sys
import types

import numpy as np

import concourse.bass as bass
import concourse.bacc as bacc
import concourse.mybir as mybir
import concourse.tile as tile
from concourse.bass_utils import run_bass_kernel_spmd


def _ensure_ntff_hook():
    """Provide antenv.axon_hooks if the image lacks it, so trace=True (or a
    caller-set BASS_TRACE=1) degrades gracefully instead of crashing in
    run_bass_kernel_spmd."""
    try:
        import antenv.axon_hooks  # noqa: F401

        return
    except ImportError:
        pass
    hook = None
    try:
        from trn_agent_boot.trn_boot import _ntff_profile_via_ctypes

        hook = _ntff_profile_via_ctypes("/opt/axon/libaxon_pjrt.so")
    except Exception:
        hook = None
    m = types.ModuleType("antenv.axon_hooks")
    m.get_axon_ntff_profile_hook = lambda: hook
    m.set_axon_ntff_profile_hook = lambda h: None
    sys.modules["antenv.axon_hooks"] = m
    try:
        import antenv

        antenv.axon_hooks = m
    except ImportError:
        pass


_ensure_ntff_hook()

F32 = mybir.dt.float32
BF16 = mybir.dt.bfloat16

N_TOK = 8192
D_MODEL = 1024
D_FF = 4096
N_EXPERTS = 8
N_CORES = 8
FCUT = D_FF // N_CORES  # 512: f-columns resident per core
NFT = FCUT // 128       # 4 f-tiles per expert per core
WCOLS = N_EXPERTS * NFT * 1024  # 32768 weight cols per tensor

LAST_EXEC_TIME_NS = None
_NC_CACHE = {}


def _blocks_from_tiles(counts):
    """Token blocks (expert, blk): each block is one expert's tokens
    (exact count, no padding anywhere), <=512 of them, split as evenly as
    possible. The very first block is 256 tokens so the first matmul
    gates on a minimal DMA."""
    blocks = []
    first = True
    for e, g in enumerate(counts):
        if g == 0:
            continue
        if first and g > 256:
            blocks.append((e, 256))
            g -= 256
            first = False
        nsub = (g + 511) // 512
        lo = g // nsub
        hi_cnt = g - lo * nsub  # hi_cnt blocks of (lo+1), rest of lo
        for i in range(nsub):
            blocks.append((e, lo + 1 if i < hi_cnt else lo))
    return blocks


def _build_nc(counts):
    counts = list(counts)
    blocks = _blocks_from_tiles(counts)
    nb = len(blocks)
    T = sum(counts)

    nc = bacc.Bacc("TRN2", target_bir_lowering=False, debug=False)
    w1 = nc.declare_dram_parameter("w1", [128, WCOLS], BF16, isOutput=False)
    w2 = nc.declare_dram_parameter("w2", [128, WCOLS], BF16, isOutput=False)
    b1 = nc.declare_dram_parameter("b1", [128, N_EXPERTS * NFT], F32, isOutput=False)
    x = nc.declare_dram_parameter("x", [128, 8 * T], BF16, isOutput=False)
    # y is stored d-major: 8 planes of [128 d, T tokens] (unscaled partials;
    # the combine weight is applied on the host during the partial sum)
    y = nc.declare_dram_parameter("y", [8, 128, T], BF16, isOutput=True)

    # per-block x column offset and token offset
    xoff = []
    toff = []
    o = t = 0
    for e, blk in blocks:
        xoff.append(o)
        toff.append(t)
        o += 8 * blk
        t += blk

    PF = 4  # x prefetch distance in blocks

    with tile.TileContext(nc) as tc:
        with (
            tc.tile_pool(name="wres", bufs=1) as wres,
            tc.tile_pool(name="const", bufs=1) as constp,
            tc.tile_pool(name="xp", bufs=PF + 1) as xp,
            tc.tile_pool(name="hp", bufs=3) as hp,
            tc.tile_pool(name="yp", bufs=6) as yp,
            tc.tile_pool(name="ps1", bufs=3, space="PSUM") as ps1,
            tc.tile_pool(name="ps2", bufs=4, space="PSUM") as ps2,
            tc.tile_pool(name="psw", bufs=1, space="PSUM") as psw,
        ):
            w1_sb = wres.tile([128, WCOLS], BF16, tag="w1")
            w2_sb = wres.tile([128, WCOLS], BF16, tag="w2")
            b1_sb = constp.tile([128, N_EXPERTS * NFT], F32, tag="b1")

            x_tiles = [None] * nb
            h_tiles = [None] * nb

            def dma_x(bi, eng=None):
                e, blk = blocks[bi]
                xt = xp.tile([128, 8 * 512], BF16, tag="x")
                (eng or nc.sync).dma_start(
                    xt[:, : 8 * blk], x[:, xoff[bi] : xoff[bi] + 8 * blk]
                )
                x_tiles[bi] = xt

            def dma_w1(e, split):
                if split:
                    for ft in range(NFT):
                        a = e * 4096 + ft * 1024
                        nc.sync.dma_start(w1_sb[:, a : a + 1024], w1[:, a : a + 1024])
                else:
                    a = e * 4096
                    nc.sync.dma_start(w1_sb[:, a : a + 4096], w1[:, a : a + 4096])

            def dma_w2(e):
                a = e * 4096
                nc.sync.dma_start(w2_sb[:, a : a + 4096], w2[:, a : a + 4096])

            # --- initial DMA schedule: gate the first matmul on w1[e0,ft0]+x[b0]
            e0 = blocks[0][0]
            loaded = {e0}
            # x[b0] on the scalar queue so it transfers in parallel with
            # w1[e0] on the sync queue -- the first matmul gates on both.
            # x[b1] is needed ~2us after the gate, so it precedes w2[e0].
            dma_x(0, eng=nc.scalar)
            dma_w1(e0, split=True)
            nc.sync.dma_start(b1_sb[:], b1[:])
            if nb > 1:
                dma_x(1)
            dma_w2(e0)
            if nb > 2:
                dma_x(2)

            # --- PE warmup: the tensor engine clock ramps over ~3us of
            # continuous execution. Burn that ramp on dummy matmuls (zeros,
            # result never read) during the initial DMA wait so the real
            # work starts at full speed. Sized to roughly fill the ~11us
            # gate window; the 128-wide fillers keep the tail granular so
            # the first real matmul is not held up.
            wu = constp.tile([128, 512], BF16, tag="wu")
            nc.vector.memset(wu[:], 0)
            pwu = psw.tile([128, 512], F32, tag="wu_ps")
            for _ in range(12):
                nc.tensor.matmul(pwu[:], wu[:, :128], wu[:], start=True, stop=True)
            for _ in range(30):
                nc.tensor.matmul(
                    pwu[:, :128], wu[:, :128], wu[:, :128], start=True, stop=True
                )

            def do_mm1(bi):
                e, blk = blocks[bi]
                x_sb = x_tiles[bi]
                h_sb = hp.tile([128, NFT * 512], BF16, tag="h")
                h_tiles[bi] = h_sb
                for ft in range(NFT):
                    ph = ps1.tile([128, 512], F32, tag="ph")
                    wa = e * 4096 + ft * 1024
                    for d in range(8):
                        nc.tensor.matmul(
                            ph[:, :blk],
                            w1_sb[:, wa + d * 128 : wa + (d + 1) * 128],
                            x_sb[:, d * blk : (d + 1) * blk],
                            start=(d == 0),
                            stop=(d == 7),
                        )
                    nc.scalar.activation(
                        h_sb[:, ft * blk : ft * blk + blk],
                        ph[:, :blk],
                        mybir.ActivationFunctionType.Silu,
                        bias=b1_sb[:, e * NFT + ft : e * NFT + ft + 1],
                    )

            def do_mm2(bi):
                # swapped roles: stationary = w2 [128f, 128d] tiles, moving =
                # h [128f, blk] -> psum [128d, blk]: cost scales with the
                # exact token count, no 128-token tile quantization.
                e, blk = blocks[bi]
                h_sb = h_tiles[bi]
                t0 = toff[bi]
                for dt in range(8):
                    py = ps2.tile([128, 512], F32, tag="py")
                    for ft in range(NFT):
                        wa = e * 4096 + ft * 1024
                        nc.tensor.matmul(
                            py[:, :blk],
                            w2_sb[:, wa + dt * 128 : wa + (dt + 1) * 128],
                            h_sb[:, ft * blk : ft * blk + blk],
                            start=(ft == 0),
                            stop=(ft == NFT - 1),
                        )
                    y_sb = yp.tile([128, 512], BF16, tag="y")
                    nc.vector.tensor_scalar_mul(y_sb[:, :blk], py[:, :blk], 1.0)
                    nc.sync.dma_start(y[dt][:, t0 : t0 + blk], y_sb[:, :blk])

            pending_w2 = []
            for bi in range(nb):
                # prefetch x (and weights on expert change) PF blocks ahead;
                # w2 is staggered one block after w1 to smooth the DMA burst.
                # Blocks 0..2 were prefetched at init, so bi=0 catches up on
                # block 3 before steady-state pf = bi+PF.
                for ee in pending_w2:
                    dma_w2(ee)
                pending_w2 = []
                pfs = [3, 4] if (bi == 0 and PF == 4) else [bi + PF]
                for pf in pfs:
                    if pf < nb:
                        ee = blocks[pf][0]
                        if ee not in loaded:
                            loaded.add(ee)
                            dma_w1(ee, split=False)
                            pending_w2.append(ee)
                        dma_x(pf)
                do_mm1(bi)
                if bi > 0:
                    do_mm2(bi - 1)
            do_mm2(nb - 1)
    nc.finalize()
    return nc


def _route(inputs, Wg, bg, k):
    """Host gate: replicate reference numerics (fp32) for routing."""
    logits = inputs.astype(np.float32) @ Wg.astype(np.float32) + bg.astype(np.float32)
    sel = np.argsort(-logits, axis=1, kind="stable")[:, :k]  # == jax.lax.top_k order
    tl = np.take_along_axis(logits, sel, axis=1).astype(np.float32)
    m = tl.max(axis=1, keepdims=True)
    e = np.exp(tl - m, dtype=np.float32)
    w = (e / e.sum(axis=1, keepdims=True)).astype(np.float32)
    return sel, w


def _prepare(inputs, W1, b1, W2, idxs, wvals, counts, blocks):
    """Build the device input arrays (shared x/cw + per-core weight cuts)."""
    import ml_dtypes

    bf16 = ml_dtypes.bfloat16
    E = N_EXPERTS
    T = sum(counts)

    xg = np.empty((T, D_MODEL), dtype=bf16)
    expert_off = []
    off = 0
    for e in range(E):
        cnt = counts[e]
        expert_off.append(off)
        xg[off : off + cnt] = inputs[idxs[e]].astype(bf16)
        off += cnt

    xcols = np.empty((128, 8 * T), dtype=bf16)
    t0 = c0 = 0
    for e, blk in blocks:
        xb = xg[t0 : t0 + blk].reshape(blk, 8, 128).transpose(2, 1, 0)
        xcols[:, c0 : c0 + 8 * blk] = xb.reshape(128, 8 * blk)
        t0 += blk
        c0 += 8 * blk

    in_maps = []
    for core in range(N_CORES):
        c0f = core * FCUT
        # (e, d, p, ft, f) -> (p, e, ft, d, f)
        w1h = np.ascontiguousarray(
            W1[:, :, c0f : c0f + FCUT]
            .astype(bf16)
            .reshape(E, 8, 128, NFT, 128)
            .transpose(2, 0, 3, 1, 4)
            .reshape(128, WCOLS)
        )
        # (e, ft, p, d) -> (p, e, ft, d)
        w2h = np.ascontiguousarray(
            W2[:, c0f : c0f + FCUT, :]
            .astype(bf16)
            .reshape(E, NFT, 128, D_MODEL)
            .transpose(2, 0, 1, 3)
            .reshape(128, WCOLS)
        )
        # (e, ft, p) -> (p, e, ft)
        b1h = np.ascontiguousarray(
            b1[:, c0f : c0f + FCUT]
            .reshape(E, NFT, 128)
            .transpose(2, 0, 1)
            .reshape(128, E * NFT)
        ).astype(np.float32)
        in_maps.append({"w1": w1h, "w2": w2h, "b1": b1h, "x": xcols})
    return in_maps, expert_off


def kernel(inputs, Wg, bg, W1, b1, W2, b2, k):
    global LAST_EXEC_TIME_NS
    k = int(np.asarray(k))
    inputs = np.ascontiguousarray(np.asarray(inputs, dtype=np.float32))
    Wg = np.asarray(Wg, dtype=np.float32)
    bg = np.asarray(bg, dtype=np.float32)
    W1 = np.asarray(W1, dtype=np.float32)
    b1 = np.asarray(b1, dtype=np.float32)
    W2 = np.asarray(W2, dtype=np.float32)
    b2 = np.asarray(b2, dtype=np.float32)

    N, D = inputs.shape
    E = Wg.shape[1]
    assert E == N_EXPERTS and D == D_MODEL and W1.shape == (E, D, D_FF)

    sel, w = _route(inputs, Wg, bg, k)

    idxs, wvals = [], []
    for e in range(E):
        tok, slot = np.nonzero(sel == e)
        idxs.append(tok)
        wvals.append(w[tok, slot])
    counts = [len(ix) for ix in idxs]
    blocks = _blocks_from_tiles(counts)

    in_maps, expert_off = _prepare(inputs, W1, b1, W2, idxs, wvals, counts, blocks)

    key = tuple(counts)
    if key not in _NC_CACHE:
        _NC_CACHE[key] = _build_nc(counts)
    nc = _NC_CACHE[key]

    trace = bool(os.environ.get("BASS_TRACE"))
    res = None
    for attempt in range(3):
        try:
            res = run_bass_kernel_spmd(
                nc, in_maps, core_ids=list(range(N_CORES)), trace=trace
            )
            break
        except Exception:
            # transient NRT/device failures recover after a short pause
            if attempt == 2:
                raise
            import time

            time.sleep(20)
    LAST_EXEC_TIME_NS = getattr(res, "exec_time_ns", None)

    T = sum(counts)
    ysum = np.zeros((8, 128, T), dtype=np.float32)
    for c in range(N_CORES):
        ysum += np.asarray(res.results[c]["y"]).astype(np.float32)
    # d-major planes [8, 128, T] -> [T, 1024]
    yT = np.ascontiguousarray(ysum.reshape(D_MODEL, T).T)

    results = np.zeros((N, D), dtype=np.float32)
    for e in range(E):
        cnt = counts[e]
        o = expert_off[e]
        # device computed silu(x W1 + b1) @ W2 unscaled; apply the combine
        # weight and the b2 term here
        results[idxs[e]] += wvals[e][:, None] * (yT[o : o + cnt] + b2[e][None, :])
    return results.astype(np.float32)


# revision 33
# speedup vs baseline: 1.1852x; 1.1852x over previous
"""MoE layer (N=8192, D=1024, F=4096, E=8, top-2) on 8 Trainium2 NeuronCores.

Strategy (F-split, fully load-balanced):
  - Host: gate (inputs @ Wg + bg), top-k selection, softmax combine weights,
    token gather per expert (the tiny O(N*D*E) part), final partial-sum
    combine + scatter-add + b2 term.
  - Device (SPMD): every core holds a 512-wide slice of the FFN hidden dim
    (F/8) of ALL 8 experts' weights resident in SBUF (bf16, 128 KB/part)
    and processes ALL token-expert pairs at 1/8 width:
        part_y = cw * (silu(x @ W1[e][:, cut]) + b1[e][cut]) @ W2[e][cut, :]
    mm2 contracts over F, so the 8 per-core partials simply sum on the
    host -- no cross-core communication, and the load is perfectly
    balanced regardless of routing (every core does identical work).

Per-core kernel layout (all flat [128, cols] SBUF tiles, bf16):
  w1: col = e*4096 + ft*1024 + d*128 + f    (stationary [128d x 128f] tiles)
  w2: col = e*4096 + ft*1024 + dcol         (stationary [128f x 128d] tiles)
  x:  per token block (<=512 tokens, single expert): col = d*blk + t
  mm1: h[f, t] = silu(sum_d w1 tile.T @ x tile + b1)    (psum [128f, blk])
  mm2: y[d, t] = sum_ft w2 tile.T @ h[ft block]         (psum [128d, blk])
  copy psum -> bf16 on VectorE, DMA out d-major ([8, 128, T] planes).
mm2 keeps tokens on the moving axis so its cost tracks the exact token
count (no 128-token tile quantization); the combine weight is applied on
the host during the partial-sum reduction. Blocks are interleaved
mm1(b+1) then mm2(b) so the PE never waits on the silu of its own block;
weights/x stream in first-use order (w2 staggered one block after w1) so
the first matmul only gates on ~1.5 MB of DMA.
"""

import math
import os
import sys
import types

import numpy as np

import concourse.bass as bass
import concourse.bacc as bacc
import concourse.mybir as mybir
import concourse.tile as tile
from concourse.bass_utils import run_bass_kernel_spmd


def _ensure_ntff_hook():
    """Provide antenv.axon_hooks if the image lacks it, so trace=True (or a
    caller-set BASS_TRACE=1) degrades gracefully instead of crashing in
    run_bass_kernel_spmd."""
    try:
        import antenv.axon_hooks  # noqa: F401

        return
    except ImportError:
        pass
    hook = None
    try:
        from trn_agent_boot.trn_boot import _ntff_profile_via_ctypes

        hook = _ntff_profile_via_ctypes("/opt/axon/libaxon_pjrt.so")
    except Exception:
        hook = None
    m = types.ModuleType("antenv.axon_hooks")
    m.get_axon_ntff_profile_hook = lambda: hook
    m.set_axon_ntff_profile_hook = lambda h: None
    sys.modules["antenv.axon_hooks"] = m
    try:
        import antenv

        antenv.axon_hooks = m
    except ImportError:
        pass


_ensure_ntff_hook()

F32 = mybir.dt.float32
BF16 = mybir.dt.bfloat16

N_TOK = 8192
D_MODEL = 1024
D_FF = 4096
N_EXPERTS = 8
N_CORES = 8
FCUT = D_FF // N_CORES  # 512: f-columns resident per core
NFT = FCUT // 128       # 4 f-tiles per expert per core
WCOLS = N_EXPERTS * NFT * 1024  # 32768 weight cols per tensor

LAST_EXEC_TIME_NS = None
_NC_CACHE = {}


def _blocks_from_tiles(counts):
    """Token blocks (expert, blk): each block is one expert's tokens
    (exact count, no padding anywhere), <=512 of them, split as evenly as
    possible. The very first block is 256 tokens so the first matmul
    gates on a minimal DMA."""
    blocks = []
    first = True
    for e, g in enumerate(counts):
        if g == 0:
            continue
        if first and g > 256:
            blocks.append((e, 256))
            g -= 256
            first = False
        nsub = (g + 511) // 512
        lo = g // nsub
        hi_cnt = g - lo * nsub  # hi_cnt blocks of (lo+1), rest of lo
        for i in range(nsub):
            blocks.append((e, lo + 1 if i < hi_cnt else lo))
    return blocks


def _build_nc(counts):
    counts = list(counts)
    blocks = _blocks_from_tiles(counts)
    nb = len(blocks)
    T = sum(counts)

    nc = bacc.Bacc("TRN2", target_bir_lowering=False, debug=False)
    w1 = nc.declare_dram_parameter("w1", [128, WCOLS], BF16, isOutput=False)
    w2 = nc.declare_dram_parameter("w2", [128, WCOLS], BF16, isOutput=False)
    b1 = nc.declare_dram_parameter("b1", [128, N_EXPERTS * NFT], F32, isOutput=False)
    x = nc.declare_dram_parameter("x", [128, 8 * T], BF16, isOutput=False)
    # y is stored d-major: 8 planes of [128 d, T tokens] (unscaled partials;
    # the combine weight is applied on the host during the partial sum)
    y = nc.declare_dram_parameter("y", [8, 128, T], BF16, isOutput=True)

    # per-block x column offset and token offset
    xoff = []
    toff = []
    o = t = 0
    for e, blk in blocks:
        xoff.append(o)
        toff.append(t)
        o += 8 * blk
        t += blk

    PF = 4  # x prefetch distance in blocks

    with tile.TileContext(nc) as tc:
        with (
            tc.tile_pool(name="wres", bufs=1) as wres,
            tc.tile_pool(name="const", bufs=1) as constp,
            tc.tile_pool(name="xp", bufs=PF + 1) as xp,
            tc.tile_pool(name="hp", bufs=3) as hp,
            tc.tile_pool(name="yp", bufs=6) as yp,
            tc.tile_pool(name="ps1", bufs=3, space="PSUM") as ps1,
            tc.tile_pool(name="ps2", bufs=4, space="PSUM") as ps2,
        ):
            w1_sb = wres.tile([128, WCOLS], BF16, tag="w1")
            w2_sb = wres.tile([128, WCOLS], BF16, tag="w2")
            b1_sb = constp.tile([128, N_EXPERTS * NFT], F32, tag="b1")

            x_tiles = [None] * nb
            h_tiles = [None] * nb

            def dma_x(bi, eng=None):
                e, blk = blocks[bi]
                xt = xp.tile([128, 8 * 512], BF16, tag="x")
                (eng or nc.sync).dma_start(
                    xt[:, : 8 * blk], x[:, xoff[bi] : xoff[bi] + 8 * blk]
                )
                x_tiles[bi] = xt

            def dma_w1(e, split):
                if split:
                    for ft in range(NFT):
                        a = e * 4096 + ft * 1024
                        nc.sync.dma_start(w1_sb[:, a : a + 1024], w1[:, a : a + 1024])
                else:
                    a = e * 4096
                    nc.sync.dma_start(w1_sb[:, a : a + 4096], w1[:, a : a + 4096])

            def dma_w2(e):
                a = e * 4096
                nc.sync.dma_start(w2_sb[:, a : a + 4096], w2[:, a : a + 4096])

            # --- initial DMA schedule: gate the first matmul on w1[e0,ft0]+x[b0]
            e0 = blocks[0][0]
            loaded = {e0}
            # x[b0] on the scalar queue so it transfers in parallel with
            # w1[e0] on the sync queue -- the first matmul gates on both.
            # x[b1] is needed ~2us after the gate, so it precedes w2[e0].
            dma_x(0, eng=nc.scalar)
            dma_w1(e0, split=True)
            nc.sync.dma_start(b1_sb[:], b1[:])
            if nb > 1:
                dma_x(1)
            dma_w2(e0)
            if nb > 2:
                dma_x(2)

            def do_mm1(bi):
                e, blk = blocks[bi]
                x_sb = x_tiles[bi]
                h_sb = hp.tile([128, NFT * 512], BF16, tag="h")
                h_tiles[bi] = h_sb
                for ft in range(NFT):
                    ph = ps1.tile([128, 512], F32, tag="ph")
                    wa = e * 4096 + ft * 1024
                    for d in range(8):
                        nc.tensor.matmul(
                            ph[:, :blk],
                            w1_sb[:, wa + d * 128 : wa + (d + 1) * 128],
                            x_sb[:, d * blk : (d + 1) * blk],
                            start=(d == 0),
                            stop=(d == 7),
                        )
                    nc.scalar.activation(
                        h_sb[:, ft * blk : ft * blk + blk],
                        ph[:, :blk],
                        mybir.ActivationFunctionType.Silu,
                        bias=b1_sb[:, e * NFT + ft : e * NFT + ft + 1],
                    )

            def do_mm2(bi):
                # swapped roles: stationary = w2 [128f, 128d] tiles, moving =
                # h [128f, blk] -> psum [128d, blk]: cost scales with the
                # exact token count, no 128-token tile quantization.
                e, blk = blocks[bi]
                h_sb = h_tiles[bi]
                t0 = toff[bi]
                for dt in range(8):
                    py = ps2.tile([128, 512], F32, tag="py")
                    for ft in range(NFT):
                        wa = e * 4096 + ft * 1024
                        nc.tensor.matmul(
                            py[:, :blk],
                            w2_sb[:, wa + dt * 128 : wa + (dt + 1) * 128],
                            h_sb[:, ft * blk : ft * blk + blk],
                            start=(ft == 0),
                            stop=(ft == NFT - 1),
                        )
                    y_sb = yp.tile([128, 512], BF16, tag="y")
                    nc.vector.tensor_scalar_mul(y_sb[:, :blk], py[:, :blk], 1.0)
                    nc.sync.dma_start(y[dt][:, t0 : t0 + blk], y_sb[:, :blk])

            pending_w2 = []
            for bi in range(nb):
                # prefetch x (and weights on expert change) PF blocks ahead;
                # w2 is staggered one block after w1 to smooth the DMA burst.
                # Blocks 0..2 were prefetched at init, so bi=0 catches up on
                # block 3 before steady-state pf = bi+PF.
                for ee in pending_w2:
                    dma_w2(ee)
                pending_w2 = []
                pfs = [3, 4] if (bi == 0 and PF == 4) else [bi + PF]
                for pf in pfs:
                    if pf < nb:
                        ee = blocks[pf][0]
                        if ee not in loaded:
                            loaded.add(ee)
                            dma_w1(ee, split=False)
                            pending_w2.append(ee)
                        dma_x(pf)
                do_mm1(bi)
                if bi > 0:
                    do_mm2(bi - 1)
            do_mm2(nb - 1)
    nc.finalize()
    return nc


def _route(inputs, Wg, bg, k):
    """Host gate: replicate reference numerics (fp32) for routing."""
    logits = inputs.astype(np.float32) @ Wg.astype(np.float32) + bg.astype(np.float32)
    sel = np.argsort(-logits, axis=1, kind="stable")[:, :k]  # == jax.lax.top_k order
    tl = np.take_along_axis(logits, sel, axis=1).astype(np.float32)
    m = tl.max(axis=1, keepdims=True)
    e = np.exp(tl - m, dtype=np.float32)
    w = (e / e.sum(axis=1, keepdims=True)).astype(np.float32)
    return sel, w


def _prepare(inputs, W1, b1, W2, idxs, wvals, counts, blocks):
    """Build the device input arrays (shared x/cw + per-core weight cuts)."""
    import ml_dtypes

    bf16 = ml_dtypes.bfloat16
    E = N_EXPERTS
    T = sum(counts)

    xg = np.empty((T, D_MODEL), dtype=bf16)
    expert_off = []
    off = 0
    for e in range(E):
        cnt = counts[e]
        expert_off.append(off)
        xg[off : off + cnt] = inputs[idxs[e]].astype(bf16)
        off += cnt

    xcols = np.empty((128, 8 * T), dtype=bf16)
    t0 = c0 = 0
    for e, blk in blocks:
        xb = xg[t0 : t0 + blk].reshape(blk, 8, 128).transpose(2, 1, 0)
        xcols[:, c0 : c0 + 8 * blk] = xb.reshape(128, 8 * blk)
        t0 += blk
        c0 += 8 * blk

    in_maps = []
    for core in range(N_CORES):
        c0f = core * FCUT
        # (e, d, p, ft, f) -> (p, e, ft, d, f)
        w1h = np.ascontiguousarray(
            W1[:, :, c0f : c0f + FCUT]
            .astype(bf16)
            .reshape(E, 8, 128, NFT, 128)
            .transpose(2, 0, 3, 1, 4)
            .reshape(128, WCOLS)
        )
        # (e, ft, p, d) -> (p, e, ft, d)
        w2h = np.ascontiguousarray(
            W2[:, c0f : c0f + FCUT, :]
            .astype(bf16)
            .reshape(E, NFT, 128, D_MODEL)
            .transpose(2, 0, 1, 3)
            .reshape(128, WCOLS)
        )
        # (e, ft, p) -> (p, e, ft)
        b1h = np.ascontiguousarray(
            b1[:, c0f : c0f + FCUT]
            .reshape(E, NFT, 128)
            .transpose(2, 0, 1)
            .reshape(128, E * NFT)
        ).astype(np.float32)
        in_maps.append({"w1": w1h, "w2": w2h, "b1": b1h, "x": xcols})
    return in_maps, expert_off


def kernel(inputs, Wg, bg, W1, b1, W2, b2, k):
    global LAST_EXEC_TIME_NS
    k = int(np.asarray(k))
    inputs = np.ascontiguousarray(np.asarray(inputs, dtype=np.float32))
    Wg = np.asarray(Wg, dtype=np.float32)
    bg = np.asarray(bg, dtype=np.float32)
    W1 = np.asarray(W1, dtype=np.float32)
    b1 = np.asarray(b1, dtype=np.float32)
    W2 = np.asarray(W2, dtype=np.float32)
    b2 = np.asarray(b2, dtype=np.float32)

    N, D = inputs.shape
    E = Wg.shape[1]
    assert E == N_EXPERTS and D == D_MODEL and W1.shape == (E, D, D_FF)

    sel, w = _route(inputs, Wg, bg, k)

    idxs, wvals = [], []
    for e in range(E):
        tok, slot = np.nonzero(sel == e)
        idxs.append(tok)
        wvals.append(w[tok, slot])
    counts = [len(ix) for ix in idxs]
    blocks = _blocks_from_tiles(counts)

    in_maps, expert_off = _prepare(inputs, W1, b1, W2, idxs, wvals, counts, blocks)

    key = tuple(counts)
    if key not in _NC_CACHE:
        _NC_CACHE[key] = _build_nc(counts)
    nc = _NC_CACHE[key]

    trace = bool(os.environ.get("BASS_TRACE"))
    res = None
    for attempt in range(3):
        try:
            res = run_bass_kernel_spmd(
                nc, in_maps, core_ids=list(range(N_CORES)), trace=trace
            )
            break
        except Exception:
            # transient NRT/device failures recover after a short pause
            if attempt == 2:
                raise
            import time

            time.sleep(20)
    LAST_EXEC_TIME_NS = getattr(res, "exec_time_ns", None)

    T = sum(counts)
    ysum = np.zeros((8, 128, T), dtype=np.float32)
    for c in range(N_CORES):
        ysum += np.asarray(res.results[c]["y"]).astype(np.float32)
    # d-major planes [8, 128, T] -> [T, 1024]
    yT = np.ascontiguousarray(ysum.reshape(D_MODEL, T).T)

    results = np.zeros((N, D), dtype=np.float32)
    for e in range(E):
        cnt = counts[e]
        o = expert_off[e]
        # device computed silu(x W1 + b1) @ W2 unscaled; apply the combine
        # weight and the b2 term here
        results[idxs[e]] += wvals[e][:, None] * (yT[o : o + cnt] + b2[e][None, :])
    return results.astype(np.float32)
